# revision 1
# baseline (speedup 1.0000x reference)
"""Depth-modulated 3x3 conv (DepthConv2d) Trainium2 Bass kernel, v2.

Math (per batch image, P = zero-padded image, k = 3i+j):
  out[o, y, x] = bias[o] + sum_{c,k} w[o,c,k] * P[c, y+i, x+j] * sim[k, y, x]
  sim[k, y, x] = exp(-8.3 * |Pd[y+i, x+j] - Pd[y, x]|)   (sim[0] == 1)

Strategy (8 cores, one image per core):
  - k=0 term is unmodulated (sim==1): pure K=64 matmul on the raw image.
  - Remaining 8 offsets form 4 pairs, each handled by one K=128 matmul
    over a [2 x 64ch] partition stack:
      A=(1,4), B=(2,5), C=(3,6) read TB1 = [P ; P-down-1-row]
      D=(7,8)            reads TB2 = [P ; P-left-1-col]
    (pair (a,b) shares one SBUF access pattern because the upper half of
    the tile holds a shifted copy of P).
  - sim is computed compactly in [y, k, x] layout (DVE diffs + ACT
    abs/exp), stored bf16, and round-tripped through DRAM into 8
    partition rows (2 per pair group) with PIXEL PAIRS PACKED as fp32
    words (2 bf16 per word).
  - Per 4-row tile, 4 replication matmuls (fp32, K=2, selection lhsT)
    run CONCURRENTLY in the PE array via tile_position row-tiling,
    broadcasting each pair's packed sim rows to 128 partitions.  fp32
    matmul copies the packed words bit-exactly, so the PSUM result can
    be read back as bf16 -- enabling the DVE 2x_1p fast mode for the
    image*sim modulate (all operands 2-byte).
  - Main matmuls are bf16; even/odd tiles share one PSUM bank via
    col-tiling (tile_position=(0,0)/(0,64)), so ACT adds bias for two
    tiles in one pass.
"""

import numpy as np
import ml_dtypes

import concourse.bass as bass
import concourse.mybir as mybir
import concourse.tile as tile
from concourse.bass_utils import run_bass_kernel_spmd

F32 = mybir.dt.float32
BF16 = mybir.dt.bfloat16
BF = ml_dtypes.bfloat16
ALPHA = 8.3
N_CORES = 8

PAIRS = [(1, 4), (2, 5), (3, 6), (7, 8)]

_WAIT_CAP = 1  # walrus engine-instruction sync-wait slot limit
_EV_CAP = 2  # InstEventSemaphore holds up to 2 waits


def _split_excess_waits(nc):
    """Move excess sync waits (>_WAIT_CAP) off engine instructions onto
    standalone InstEventSemaphore carriers inserted just before, on the same
    engine.  Tile's scheduler often leaves 2+ waits on one instruction,
    which walrus codegen rejects ("Too many sync wait commands")."""
    import bass_rust

    for bb in nc.main_func.blocks:
        out = []
        for ins in bb.instructions:
            si = ins.sync_info
            tname = type(ins).__name__
            if (
                si is not None
                and si.on_wait
                and len(si.on_wait) > _WAIT_CAP
                and tname != "InstEventSemaphore"
            ):
                waits = list(si.on_wait)
                keep = waits[-_WAIT_CAP:]
                excess = waits[:-_WAIT_CAP]
                while excess:
                    chunk, excess = excess[:_EV_CAP], excess[_EV_CAP:]
                    ev = bass_rust.InstEventSemaphore(
                        name=nc.get_next_instruction_name(), ins=[], outs=[]
                    )
                    ev.engine = ins.engine
                    ev.sync_info = bass_rust.SyncInfo(on_wait=chunk, on_update=[])
                    out.append(ev)
                si.on_wait = keep
            out.append(ins)
        bb.instructions[:] = out


def build_bass(loop_reps: int = 0, probe: str = "", split_waits: bool = True):
    nc = bass.Bass()
    img_d = nc.dram_tensor("image", [64, 128, 128], BF16, kind="ExternalInput")
    dep_d = nc.dram_tensor("depth", [128, 128], F32, kind="ExternalInput")
    wp_d = nc.dram_tensor("wp", [128, 4, 64], BF16, kind="ExternalInput")
    w0_d = nc.dram_tensor("w0", [128, 64], BF16, kind="ExternalInput")
    sel_d = nc.dram_tensor("sel", [128, 128], F32, kind="ExternalInput")
    bias_d = nc.dram_tensor("bias", [128, 1], F32, kind="ExternalInput")
    out_d = nc.dram_tensor("out", [64, 128, 128], BF16, kind="ExternalOutput")

    with tile.TileContext(nc) as tc:
        with (
            tc.tile_pool(name="singles", bufs=1) as singles,
            tc.tile_pool(name="dram", bufs=1, space="DRAM") as drampool,
            tc.tile_pool(name="reps", bufs=1, space="PSUM") as repsp,
            tc.tile_pool(name="outps", bufs=3, space="PSUM") as outps,
            tc.tile_pool(name="mods", bufs=10) as mods,
            tc.tile_pool(name="outsb", bufs=3) as outsb,
            tc.tile_pool(name="stage", bufs=4) as stagep,
        ):
            # ---- image arrangements, 4 row blocks of 34 padded rows.
            # TB1: lower = P (padded), upper = P shifted down 1 row.
            # TB2: lower = P,          upper = U = P shifted left 1 col.
            def load_p_layer(tb, p0, b, shift, cols):
                # tile[p0:p0+64, r, cols] = P[32b + r + shift, :] for valid r
                lo = max(0, 1 - 32 * b - shift)
                hi = min(33, 128 - 32 * b - shift)
                if lo > 0:
                    nc.vector.memset(tb[p0 : p0 + 64, 0:lo, :], 0.0)
                if hi < 33:
                    nc.vector.memset(tb[p0 : p0 + 64, hi + 1 : 34, :], 0.0)
                nc.sync.dma_start(
                    out=tb[p0 : p0 + 64, lo : hi + 1, cols : cols + 128],
                    in_=img_d[:, 32 * b + lo + shift - 1 : 32 * b + hi + shift, :],
                )

            TB1 = []
            for b in range(4):
                t1 = singles.tile([128, 34, 130], BF16, name=f"TB1_{b}")
                nc.vector.memset(t1[:, :, 0:1], 0.0)
                nc.vector.memset(t1[:, :, 129:130], 0.0)
                load_p_layer(t1, 0, b, 0, 1)  # P
                load_p_layer(t1, 64, b, 1, 1)  # P down 1 row
                TB1.append(t1)

            # ---- depth rows, partition-shifted copies D_i[y, X] = Pd[y+i, X]
            D0 = singles.tile([128, 130], F32)
            D1 = singles.tile([128, 130], F32)
            D2 = singles.tile([128, 130], F32)
            nc.vector.memset(D0[:, :], 0.0)
            nc.vector.memset(D1[:, :], 0.0)
            nc.vector.memset(D2[:, :], 0.0)
            nc.sync.dma_start(out=D0[1:128, 1:129], in_=dep_d[0:127, :])
            nc.sync.dma_start(out=D1[0:128, 1:129], in_=dep_d[:, :])
            nc.sync.dma_start(out=D2[0:127, 1:129], in_=dep_d[1:128, :])
            Ds = [D0, D1, D2]

            # ---- weights / selection / bias ----
            wp_sb = singles.tile([128, 4, 64], BF16)
            w0_sb = singles.tile([128, 64], BF16)
            sel_sb = singles.tile([128, 128], F32)
            bias_sb = singles.tile([128, 1], F32)
            nc.sync.dma_start(out=wp_sb[:], in_=wp_d[:])
            nc.sync.dma_start(out=w0_sb[:], in_=w0_d[:])
            nc.sync.dma_start(out=sel_sb[:], in_=sel_d[:])
            nc.sync.dma_start(out=bias_sb[:], in_=bias_d[:])

            # ---- sim (k=1..8) in [y, k, x]: exp(-a*|D_i[y,x+j] - D0[y,x]|)
            simf = singles.tile([128, 8, 128], F32)
            for k in range(1, 9):
                i, j = k // 3, k % 3
                nc.vector.tensor_tensor(
                    out=simf[:, k - 1, :],
                    in0=Ds[i][:, j : j + 128],
                    in1=D0[:, 0:128],
                    op=mybir.AluOpType.subtract,
                )
            nc.scalar.activation(
                out=simf[:, :, :],
                in_=simf[:, :, :],
                func=mybir.ActivationFunctionType.Abs,
                scale=ALPHA,
            )
            sim_bf = singles.tile([128, 8, 128], BF16)
            nc.scalar.activation(
                out=sim_bf[:, :, :],
                in_=simf[:, :, :],
                func=mybir.ActivationFunctionType.Exp,
                scale=-1.0,
            )
            # flatten [y, k, x] -> per-pair-group packed rows through DRAM:
            # partition 32g+r holds sim[k_gr] over 16384 px as 8192 fp32
            # words (bf16 pixel pairs).
            spw_sb = singles.tile([128, 16384], BF16)
            sim_dram = drampool.tile([8, 16384], BF16)
            nc.sync.dma_start(
                out=sim_dram[:].rearrange("k (y x) -> y k x", y=128),
                in_=sim_bf[:],
            )
            for g, (a, b2) in enumerate(PAIRS):
                step = b2 - a
                nc.sync.dma_start(
                    out=spw_sb[32 * g : 32 * g + 2, :],
                    in_=sim_dram[a - 1 : b2 : step, :],
                )
            selb_sb = singles.tile([128, 128], BF16)
            nc.vector.tensor_copy(out=selb_sb[:], in_=sel_sb[:])

            # ---- main loop: 32 tiles of 4 image rows (512 px each) ----
            import contextlib

            gps_d = probe == "gpsd"
            ab = probe[3:] if probe.startswith("ab_") else ""
            # ablation static data
            if ab in ("norep", "static"):
                statps = []
                for g in range(4):
                    sp_t = repsp.tile([128, 8, 64], F32, tag=f"rep{g}", name=f"statrep{g}")
                    nc.tensor.matmul(
                        out=sp_t[:],
                        lhsT=sel_sb[32 * g : 32 * g + 2, :],
                        rhs=spw_sb[32 * g : 32 * g + 2, 0:512],
                        start=True,
                        stop=True,
                        tile_position=(32 * g, 0),
                    )
                    statps.append(sp_t)
            if ab == "static":
                statmods = []
                for g in range(4):
                    sm = singles.tile([128, 4, 128], BF16, name=f"statmod{g}")
                    nc.vector.memset(sm[:], 0.25)
                    statmods.append(sm)

            loop_ctx = (
                tc.For_i(0, loop_reps, 1) if loop_reps else contextlib.nullcontext()
            )
            ntiles = 32
            skips = set()
            if probe.startswith("mini"):
                parts = probe[4:].split(":")
                ntiles = int(parts[0] or 2)
                skips = set(parts[1:])
            with loop_ctx:
              for t in range(ntiles):
                y0 = 4 * t
                b = t // 8
                yl = y0 - 32 * b
                T1 = TB1[b]
                even = t % 2 == 0
                cpos = 0 if even else 64
                if probe == "sepout":
                    even, cpos = True, 0

                repAB = repsp.tile([128, 2, 4, 128], F32, tag="rAB", name=f"rAB{t}")
                repCD = repsp.tile([128, 2, 4, 128], F32, tag="rCD", name=f"rCD{t}")
                for g in range(4):
                    dst = repAB[:, g] if g < 2 else repCD[:, g - 2]
                    nc.tensor.matmul(
                        out=dst,
                        lhsT=selb_sb[32 * g : 32 * g + 2, :],
                        rhs=spw_sb[
                            32 * g : 32 * g + 2, 128 * y0 : 128 * y0 + 512
                        ].rearrange("p (r x) -> p r x", r=4),
                        start=True,
                        stop=True,
                        tile_position=(32 * g, 0),
                    )

                # modulates: mod = image_slice * sim_pair (bf16, 2x_1p)
                modAB = mods.tile([128, 2, 4, 128], BF16, tag="modAB", name=f"mAB{t}")
                modC = mods.tile([128, 4, 128], BF16, tag="modC", name=f"mC{t}")
                modD = mods.tile([128, 4, 128], BF16, tag="modD", name=f"mD{t}")
                modA = modAB[:, 0]
                modB = modAB[:, 1]
                nc.vector.tensor_tensor(
                    out=modAB[:, 0],
                    in0=T1[:, yl : yl + 4, 1:129],
                    in1=repAB[:, 0],
                    op=mybir.AluOpType.mult,
                )
                nc.vector.tensor_tensor(
                    out=modAB[:, 1],
                    in0=T1[:, yl : yl + 4, 2:130],
                    in1=repAB[:, 1],
                    op=mybir.AluOpType.mult,
                )
                nc.vector.tensor_tensor(
                    out=modC[:],
                    in0=T1[:, yl + 1 : yl + 5, 0:128],
                    in1=repCD[:, 0],
                    op=mybir.AluOpType.mult,
                )
                stg = stagep.tile([128, 4, 128], BF16, tag="stg", name=f"st{t}")
                nc.scalar.activation(
                    out=stg[:],
                    in_=repCD[:, 1],
                    func=mybir.ActivationFunctionType.Copy,
                )
                nc.gpsimd.tensor_tensor(
                    out=modD[0:64],
                    in0=T1[0:64, yl + 2 : yl + 6, 1:129],
                    in1=stg[0:64],
                    op=mybir.AluOpType.mult,
                )
                nc.gpsimd.tensor_tensor(
                    out=modD[64:128],
                    in0=T1[64:128, yl + 1 : yl + 5, 2:130],
                    in1=stg[64:128],
                    op=mybir.AluOpType.mult,
                )

                if ab == "nomain":
                    continue
                if even:
                    out_ps = outps.tile([128, 4, 128], F32, tag="outps", name=f"o{t}")
                out_lo, out_hi = (0, 64) if even else (64, 128)
                for gi, mod in enumerate([modA, modB, modC, modD]):
                    nc.tensor.matmul(
                        out=out_ps[out_lo:out_hi],
                        lhsT=wp_sb[:, gi, :],
                        rhs=mod[:],
                        start=(gi == 0),
                        stop=False,
                        tile_position=(0, cpos),
                    )
                # unmodulated k=0 term: raw P rows y0..y0+3
                if even:
                    k0_rhs = T1[0:64, yl : yl + 4, 0:128]
                else:
                    k0_rhs = T1[64:128, yl - 1 : yl + 3, 0:128]
                nc.tensor.matmul(
                    out=out_ps[out_lo:out_hi],
                    lhsT=w0_sb[out_lo:out_hi, :],
                    rhs=k0_rhs,
                    start=False,
                    stop=True,
                    tile_position=(cpos, cpos),
                )

                if not even:
                    if t % 16 == 1:
                        out_sb = outsb.tile(
                            [128, 8, 4, 128], BF16, tag="outsb", name=f"os{t}"
                        )
                    q = (t // 2) % 8
                    nc.scalar.activation(
                        out=out_sb[:, q, :, :],
                        in_=out_ps[:],
                        func=mybir.ActivationFunctionType.Identity,
                        bias=bias_sb[:, 0:1],
                        scale=1.0,
                    )
                    if t % 16 == 15 and ab != "noout":
                        yb = y0 - 60  # first out row of this 16-tile batch
                        batch = out_d[:, yb : yb + 64, :].rearrange(
                            "c (q h r) x -> c q h r x", q=8, h=2
                        )
                        nc.sync.dma_start(out=batch[:, :, 0, :, :], in_=out_sb[0:64])
                        nc.sync.dma_start(out=batch[:, :, 1, :, :], in_=out_sb[64:128])

    if split_waits:
        _split_excess_waits(nc)
    return nc


_NC_CACHE = None


def _get_nc():
    global _NC_CACHE
    if _NC_CACHE is None:
        _NC_CACHE = build_bass()
    return _NC_CACHE


def _prep_operands(weight, bias):
    wtk = weight.reshape(64, 64, 9)  # w[o, c, k]
    wp = np.zeros((128, 4, 64), BF)
    for g, (a, b) in enumerate(PAIRS):
        wp[0:64, g, :] = wtk[:, :, a].T.astype(BF)
        wp[64:128, g, :] = wtk[:, :, b].T.astype(BF)
    w0 = np.zeros((128, 64), BF)
    w0[0:64] = wtk[:, :, 0].T.astype(BF)
    w0[64:128] = wtk[:, :, 0].T.astype(BF)
    sel = np.zeros((128, 128), np.float32)
    for g in range(4):
        sel[32 * g, 0:64] = 1.0
        sel[32 * g + 1, 64:128] = 1.0
    bias2 = np.concatenate([bias, bias]).reshape(128, 1).astype(np.float32)
    return wp, w0, sel, bias2


def kernel(image, depth, weight, bias, **kwargs):
    image = np.asarray(image, dtype=np.float32)
    depth = np.ascontiguousarray(np.asarray(depth, dtype=np.float32))
    weight = np.ascontiguousarray(np.asarray(weight, dtype=np.float32))
    bias = np.ascontiguousarray(np.asarray(bias, dtype=np.float32))

    B = image.shape[0]
    assert B == N_CORES, f"expected batch {N_CORES}, got {B}"

    wp, w0, sel, bias2 = _prep_operands(weight, bias)
    image_bf = image.astype(BF)

    global _last_in_maps
    nc = _get_nc()
    in_maps = [
        {
            "image": np.ascontiguousarray(image_bf[b]),
            "depth": depth[b, 0],
            "wp": wp,
            "w0": w0,
            "sel": sel,
            "bias": bias2,
        }
        for b in range(B)
    ]
    _last_in_maps = in_maps
    res = run_bass_kernel_spmd(nc, in_maps, core_ids=list(range(N_CORES)))
    out = np.stack([r["out"] for r in res.results], axis=0)
    return out.astype(np.float32)



# revision 37
# speedup vs baseline: 1.3478x; 1.3478x over previous
"""Depth-modulated 3x3 conv (DepthConv2d) Trainium2 Bass kernel, v3.

Math (per batch image, P = zero-padded image, k = 3i+j):
  out[o, y, x] = bias[o] + sum_{c,k} w[o,c,k] * P[c, y+i, x+j] * sim[k, y, x]
  sim[k, y, x] = exp(-8.3 * |Pd[y+i, x+j] - Pd[y, x]|)   (sim[0] == 1)

Strategy (8 cores, one image per core):
  - Offsets paired for K=128 matmuls with 4B-aligned, fully bf16 DVE
    modulates:
      Ta = [img<<1 down-right pad ; img pad] stacks, serving pairs
      (1,2), (4,5), (7,8) at row offsets g=0,1,2 -- ONE fused DVE
      tensor_tensor (2x_1p mode) modulates all three groups;
      Tb serves pair (3,6) plus the unmodulated k=0 term (K=64 matmul).
  - sim rows are packed as bf16 PIXEL PAIRS inside fp32 words; the
    replication matmul (fp32, K=2, 1.0/0.0 selection lhsT, row-tiled
    over 4 PE row groups) bit-copies the words into PSUM, which the DVE
    then reads back as bf16 (2x_1p fast mode) for the modulates.
  - Host pre-pads the image into three shifted plane-contiguous copies
    so every image DMA moves ~9KB contiguous descriptor runs; output is
    dumped partition-major and reassembled on host.
  - Main matmuls bf16; even/odd tiles share one PSUM bank via
    col-tiling (tile_position=(0,0)/(0,64)); ACT adds bias per odd tile.
"""

import numpy as np
import ml_dtypes

import concourse.bass as bass
import concourse.mybir as mybir
import concourse.tile as tile
from concourse.bass_utils import run_bass_kernel_spmd
import bass_rust

F32 = mybir.dt.float32
BF16 = mybir.dt.bfloat16
BF = ml_dtypes.bfloat16
ALPHA = 8.3
N_CORES = 8

_WAIT_CAP = 1  # walrus engine-instruction sync-wait slot limit
_EV_CAP = 2  # InstEventSemaphore holds up to 2 waits


def _split_excess_waits(nc):
    """Move excess sync waits (>_WAIT_CAP) off engine instructions onto
    standalone InstEventSemaphore carriers inserted just before, on the same
    engine.  Tile's scheduler often leaves 2+ waits on one instruction,
    which walrus codegen rejects ("Too many sync wait commands")."""
    for bb in nc.main_func.blocks:
        out = []
        for ins in bb.instructions:
            si = ins.sync_info
            tname = type(ins).__name__
            if (
                si is not None
                and si.on_wait
                and len(si.on_wait) > _WAIT_CAP
                and tname != "InstEventSemaphore"
            ):
                waits = list(si.on_wait)
                keep = waits[-_WAIT_CAP:]
                excess = waits[:-_WAIT_CAP]
                while excess:
                    chunk, excess = excess[:_EV_CAP], excess[_EV_CAP:]
                    ev = bass_rust.InstEventSemaphore(
                        name=nc.get_next_instruction_name(), ins=[], outs=[]
                    )
                    ev.engine = ins.engine
                    ev.sync_info = bass_rust.SyncInfo(on_wait=chunk, on_update=[])
                    out.append(ev)
                si.on_wait = keep
            out.append(ins)
        bb.instructions[:] = out


def _ov(apobj, pattern):
    """Copy an AP and overwrite its access pattern ([stride, num] pairs,
    partition dim first).  Used for overlapping read windows the slicing
    API cannot express."""
    c = apobj.copy()
    c.ap = bass_rust.VecI64Pair([list(p) for p in pattern])
    return c


def build_bass(
    split_waits: bool = True, rings: str = "cgg", ntiles: int = 32, skip: tuple = ()
):
    """rings: 3 chars for (small-loads ring, Tb/spw alt ring, spw alt ring):
    's'=sync, 'c'=scalar, 'g'=gpsimd."""
    nc = bass.Bass()

    def _eng(ch):
        return {"s": nc.sync, "c": nc.scalar, "g": nc.gpsimd}[ch]
    # imgpad planes (bf16, plane-contiguous):
    #   plane 0 (L): L[r, x] = P[r-1, x-1]   (img at [2:130, 2:130])
    #   plane 1 (U): U[r, x] = P[r-1, x]     (img at [2:130, 1:129])
    #   plane 2 (D): D[r, x] = P[r,   x]     (img at [1:129, 1:129])
    img_d = nc.dram_tensor("imgpad", [3, 64, 133, 132], BF16, kind="ExternalInput")
    # dpad3[y, i, x] = Pd[y+i, x]  (padded depth rows, f32)
    dep_d = nc.dram_tensor("dpad3", [128, 3, 130], F32, kind="ExternalInput")
    wp_d = nc.dram_tensor("wp", [128, 4, 64], BF16, kind="ExternalInput")
    w0_d = nc.dram_tensor("w0", [64, 64], BF16, kind="ExternalInput")
    sel_d = nc.dram_tensor("sel", [128, 128], F32, kind="ExternalInput")
    bias_d = nc.dram_tensor("bias", [128, 1], F32, kind="ExternalInput")
    out_d = nc.dram_tensor("out", [128, 2, 4096], BF16, kind="ExternalOutput")

    with tile.TileContext(nc) as tc:
        with (
            tc.tile_pool(name="singles", bufs=1) as singles,
            tc.tile_pool(name="dram", bufs=1, space="DRAM") as drampool,
            tc.tile_pool(name="repp", bufs=1, space="PSUM") as repp,
            tc.tile_pool(name="outps", bufs=3, space="PSUM") as outps,
            tc.tile_pool(name="mods", bufs=4) as mods,
            tc.tile_pool(name="outsb", bufs=2) as outsb,
        ):
            e1, e2, e3 = _eng(rings[0]), _eng(rings[1]), _eng(rings[2])
            # ---- depth + small operands on ring e1 first
            Dt = singles.tile([128, 3, 130], F32)
            e1.dma_start(out=Dt[:], in_=dep_d[:])
            wp_sb = singles.tile([128, 4, 64], BF16)
            w0_sb = singles.tile([64, 64], BF16)
            sel_sb = singles.tile([128, 128], F32)
            bias_sb = singles.tile([128, 1], F32)
            e1.dma_start(out=wp_sb[:], in_=wp_d[:])
            e1.dma_start(out=w0_sb[:], in_=w0_d[:])
            e1.dma_start(out=sel_sb[:], in_=sel_d[:])
            e1.dma_start(out=bias_sb[:], in_=bias_d[:])

            # ---- image stacks.
            # Ta[p<64]  = L rows 32b+1.. : P[32b+r, x-1] (j=1 view at x0=2)
            # Ta[p>=64] = U rows 32b+1.. : P[32b+r, x]   (j=2 view at x0=2)
            # Tb[p<64]  = U rows 32b+1.. : P[32b+r, x]   (k3/k0 at x0=0)
            # Tb[p>=64] = D rows 32b+1.. : P[32b+r+1, x] (k6 at x0=0)
            Ta = singles.tile([128, 4, 34, 132], BF16, name="Ta")
            Tb = singles.tile([128, 4, 34, 132], BF16, name="Tb")
            for b in range(4):
                nc.sync.dma_start(
                    out=Ta[:, b],
                    in_=img_d[0:2, :, 32 * b + 1 : 32 * b + 35, :].rearrange(
                        "h c r x -> (h c) r x"
                    ),
                )
                eng = nc.sync if b % 2 else e2
                eng.dma_start(
                    out=Tb[:, b],
                    in_=img_d[1:3, :, 32 * b + 1 : 32 * b + 35, :].rearrange(
                        "h c r x -> (h c) r x"
                    ),
                )

            # ---- sim (k=1..8) in [y, k, x]: exp(-a*|Pd[y+i,x+j] - Pd[y,x]|)
            simf = singles.tile([128, 8, 128], F32)
            for k in range(1, 9):
                i, j = k // 3, k % 3
                nc.vector.tensor_tensor(
                    out=simf[:, k - 1, :],
                    in0=Dt[:, i, j : j + 128],
                    in1=Dt[:, 0, 0:128],
                    op=mybir.AluOpType.subtract,
                )
            nc.scalar.activation(
                out=simf[:, :, :],
                in_=simf[:, :, :],
                func=mybir.ActivationFunctionType.Abs,
                scale=ALPHA,
            )
            sim_bf = singles.tile([128, 8, 128], BF16)
            nc.scalar.activation(
                out=sim_bf[:, :, :],
                in_=simf[:, :, :],
                func=mybir.ActivationFunctionType.Exp,
                scale=-1.0,
            )
            # spw rows (bf16 pixel pairs packed in fp32 words):
            #   32g+0 = sim for lower half, 32g+1 = upper half
            #   g<3: k = 3g+1+m ; g=3: k = 3, 6
            spw = singles.tile([128, 8192], F32)
            spw_bf = spw[:].bitcast(BF16)  # [128, 16384]
            sim_dram = drampool.tile([8, 16384], BF16)
            e1.dma_start(
                out=sim_dram[:].rearrange("k (y x) -> y k x", y=128),
                in_=sim_bf[:],
            )
            for g in range(4):
                if g < 3:
                    src = sim_dram[3 * g : 3 * g + 2, :]  # k = 3g+1, 3g+2
                else:
                    src = sim_dram[2:6:3, :]  # k = 3, 6
                eng = [e1, e3, e1, e3][g]
                eng.dma_start(out=spw_bf[32 * g : 32 * g + 2, :], in_=src)

            # ---- main loop: 32 tiles of 4 image rows (512 px each) ----
            for t in range(ntiles):
                b = t // 8
                yl = 4 * t - 32 * b
                even = t % 2 == 0
                cpos = 0 if even else 64

                # one full PSUM bank per rep matmul (concurrent row-tiled
                # matmuls must not share a bank)
                rep = repp.tile([128, 4, 512], F32, tag="rep", name=f"rep{t}")
                for g in range(4):
                    if "rep" in skip:
                        continue
                    nc.tensor.matmul(
                        out=rep[:, g, 0:256],
                        lhsT=sel_sb[32 * g : 32 * g + 2, :],
                        rhs=spw[32 * g : 32 * g + 2, 256 * t : 256 * t + 256],
                        start=True,
                        stop=True,
                        tile_position=(32 * g, 0),
                    )
                rep_bf = rep[:].bitcast(BF16)  # [128, 4, 1024], first 512 valid

                m = mods.tile([128, 4, 4, 128], BF16, tag="mod", name=f"m{t}")
                # fused modulate for groups 0..2: in0 rows yl+g+rho, x0=2
                a0 = Ta[:, b, yl : yl + 4, 2:130]
                es = 4 * 34 * 132  # elements per partition in Ta
                base_pat = [
                    [es, 128],
                    [132, 3],
                    [132, 4],
                    [1, 128],
                ]
                if "mod" not in skip:
                    nc.vector.tensor_tensor(
                        out=m[:, 0:3],
                        in0=_ov(a0, base_pat),
                        in1=rep_bf[:, 0:3, 0:512].rearrange(
                            "p g (r x) -> p g r x", r=4
                        ),
                        op=mybir.AluOpType.mult,
                    )
                    nc.vector.tensor_tensor(
                        out=m[:, 3],
                        in0=Tb[:, b, yl + 1 : yl + 5, 0:128],
                        in1=rep_bf[:, 3, 0:512].rearrange("p (r x) -> p r x", r=4),
                        op=mybir.AluOpType.mult,
                    )
                else:
                    nc.vector.memset(m[:], 0.25)

                if even:
                    out_ps = outps.tile([128, 4, 128], F32, tag="outps", name=f"o{t}")
                out_lo, out_hi = (0, 64) if even else (64, 128)
                for g in range(4):
                    if "main" in skip:
                        continue
                    nc.tensor.matmul(
                        out=out_ps[out_lo:out_hi],
                        lhsT=wp_sb[:, g, :],
                        rhs=m[:, g],
                        start=(g == 0),
                        stop=False,
                        tile_position=(0, cpos),
                    )
                # unmodulated k=0 term: P rows yl.. on Tb lower half
                nc.tensor.matmul(
                    out=out_ps[out_lo:out_hi],
                    lhsT=w0_sb[:, :],
                    rhs=Tb[0:64, b, yl : yl + 4, 0:128],
                    start=("main" in skip),
                    stop=True,
                    tile_position=(0, cpos),
                )

                if not even:
                    if t % 16 == 1:
                        out_sb = outsb.tile(
                            [128, 8, 4, 128], BF16, tag="outsb", name=f"os{t}"
                        )
                    q = (t // 2) % 8
                    nc.scalar.activation(
                        out=out_sb[:, q, :, :],
                        in_=out_ps[:],
                        func=mybir.ActivationFunctionType.Identity,
                        bias=bias_sb[:, 0:1],
                        scale=1.0,
                    )
                    if t % 16 == 15:
                        nc.sync.dma_start(
                            out=out_d[:, t // 16, :],
                            in_=out_sb[:].rearrange("p q r x -> p (q r x)"),
                        )

    if split_waits:
        _split_excess_waits(nc)
    return nc


_NC_CACHE = None


def _get_nc():
    global _NC_CACHE
    if _NC_CACHE is None:
        _NC_CACHE = build_bass()
    return _NC_CACHE


def _prep_operands(weight, bias):
    wtk = weight.reshape(64, 64, 9)  # w[o, c, k]
    wp = np.zeros((128, 4, 64), BF)
    for g in range(3):
        wp[0:64, g, :] = wtk[:, :, 3 * g + 1].T.astype(BF)  # lower = j=1
        wp[64:128, g, :] = wtk[:, :, 3 * g + 2].T.astype(BF)  # upper = j=2
    wp[0:64, 3, :] = wtk[:, :, 3].T.astype(BF)  # lower = k3
    wp[64:128, 3, :] = wtk[:, :, 6].T.astype(BF)  # upper = k6
    w0 = np.ascontiguousarray(wtk[:, :, 0].T).astype(BF)  # [c, o]
    sel = np.zeros((128, 128), np.float32)
    for g in range(4):
        sel[32 * g, 0:64] = 1.0
        sel[32 * g + 1, 64:128] = 1.0
    bias2 = np.concatenate([bias, bias]).reshape(128, 1).astype(np.float32)
    return wp, w0, sel, bias2


def _prep_image(img_bf):
    """img_bf: [64, 128, 128] bf16 -> padded shifted planes [3, 64, 133, 132]."""
    Z = np.zeros((3, 64, 133, 132), BF)
    Z[0, :, 2:130, 2:130] = img_bf  # L: P[r-1, x-1]
    Z[1, :, 2:130, 1:129] = img_bf  # U: P[r-1, x]
    Z[2, :, 1:129, 1:129] = img_bf  # D: P[r, x]
    return Z


def _prep_depth(dep):
    """dep: [128, 128] f32 -> dpad3[y, i, x] = Pd[y+i, x], [128, 3, 130]."""
    Pd = np.zeros((131, 130), np.float32)
    Pd[1:129, 1:129] = dep
    out = np.empty((128, 3, 130), np.float32)
    for i in range(3):
        out[:, i, :] = Pd[i : i + 128, :]
    return out


def kernel(image, depth, weight, bias, **kwargs):
    image = np.asarray(image, dtype=np.float32)
    depth = np.ascontiguousarray(np.asarray(depth, dtype=np.float32))
    weight = np.ascontiguousarray(np.asarray(weight, dtype=np.float32))
    bias = np.ascontiguousarray(np.asarray(bias, dtype=np.float32))

    B = image.shape[0]
    assert B == N_CORES, f"expected batch {N_CORES}, got {B}"

    wp, w0, sel, bias2 = _prep_operands(weight, bias)
    image_bf = image.astype(BF)

    global _last_in_maps
    nc = _get_nc()
    in_maps = [
        {
            "imgpad": _prep_image(image_bf[b]),
            "dpad3": _prep_depth(depth[b, 0]),
            "wp": wp,
            "w0": w0,
            "sel": sel,
            "bias": bias2,
        }
        for b in range(B)
    ]
    _last_in_maps = in_maps
    res = run_bass_kernel_spmd(nc, in_maps, core_ids=list(range(N_CORES)))
    # out[p, B2, q, r, x]: p = h*64 + c (h=0 even tile, h=1 odd);
    # image row = 64*B2 + 8*q + 4*h + r
    outs = []
    for r in res.results:
        v = r["out"].reshape(2, 64, 2, 8, 4, 128)  # h c B2 q r x
        full = np.transpose(v, (1, 2, 3, 0, 4, 5)).reshape(64, 128, 128)
        outs.append(full)
    return np.stack(outs, axis=0).astype(np.float32)


# revision 40
# speedup vs baseline: 1.6962x; 1.2585x over previous
"""Depth-modulated 3x3 conv (DepthConv2d) Trainium2 Bass kernel, v3.

Math (per batch image, P = zero-padded image, k = 3i+j):
  out[o, y, x] = bias[o] + sum_{c,k} w[o,c,k] * P[c, y+i, x+j] * sim[k, y, x]
  sim[k, y, x] = exp(-8.3 * |Pd[y+i, x+j] - Pd[y, x]|)   (sim[0] == 1)

Strategy (8 cores, one image per core):
  - Offsets paired for K=128 matmuls with 4B-aligned, fully bf16 DVE
    modulates:
      Ta = [img<<1 down-right pad ; img pad] stacks, serving pairs
      (1,2), (4,5), (7,8) at row offsets g=0,1,2 -- ONE fused DVE
      tensor_tensor (2x_1p mode) modulates all three groups;
      Tb serves pair (3,6) plus the unmodulated k=0 term (K=64 matmul).
  - sim rows are packed as bf16 PIXEL PAIRS inside fp32 words; the
    replication matmul (fp32, K=2, 1.0/0.0 selection lhsT, row-tiled
    over 4 PE row groups) bit-copies the words into PSUM, which the DVE
    then reads back as bf16 (2x_1p fast mode) for the modulates.
  - Host pre-pads the image into three shifted plane-contiguous copies
    so every image DMA moves ~9KB contiguous descriptor runs; output is
    dumped partition-major and reassembled on host.
  - Main matmuls bf16; even/odd tiles share one PSUM bank via
    col-tiling (tile_position=(0,0)/(0,64)); ACT adds bias per odd tile.
"""

import numpy as np
import ml_dtypes

import concourse.bass as bass
import concourse.mybir as mybir
import concourse.tile as tile
from concourse.bass_utils import run_bass_kernel_spmd
import bass_rust

F32 = mybir.dt.float32
BF16 = mybir.dt.bfloat16
BF = ml_dtypes.bfloat16
ALPHA = 8.3
N_CORES = 8

_WAIT_CAP = 1  # walrus engine-instruction sync-wait slot limit
_EV_CAP = 2  # InstEventSemaphore holds up to 2 waits


def _split_excess_waits(nc):
    """Move excess sync waits (>_WAIT_CAP) off engine instructions onto
    standalone InstEventSemaphore carriers inserted just before, on the same
    engine.  Tile's scheduler often leaves 2+ waits on one instruction,
    which walrus codegen rejects ("Too many sync wait commands")."""
    for bb in nc.main_func.blocks:
        out = []
        for ins in bb.instructions:
            si = ins.sync_info
            tname = type(ins).__name__
            if (
                si is not None
                and si.on_wait
                and len(si.on_wait) > _WAIT_CAP
                and tname != "InstEventSemaphore"
            ):
                waits = list(si.on_wait)
                keep = waits[-_WAIT_CAP:]
                excess = waits[:-_WAIT_CAP]
                while excess:
                    chunk, excess = excess[:_EV_CAP], excess[_EV_CAP:]
                    ev = bass_rust.InstEventSemaphore(
                        name=nc.get_next_instruction_name(), ins=[], outs=[]
                    )
                    ev.engine = ins.engine
                    ev.sync_info = bass_rust.SyncInfo(on_wait=chunk, on_update=[])
                    out.append(ev)
                si.on_wait = keep
            out.append(ins)
        bb.instructions[:] = out


def _ov(apobj, pattern):
    """Copy an AP and overwrite its access pattern ([stride, num] pairs,
    partition dim first).  Used for overlapping read windows the slicing
    API cannot express."""
    c = apobj.copy()
    c.ap = bass_rust.VecI64Pair([list(p) for p in pattern])
    return c


def build_bass(
    split_waits: bool = True, rings: str = "css", ntiles: int = 32, skip: tuple = ()
):
    """rings: 3 chars for (small-loads ring, Tb/spw alt ring, spw alt ring):
    's'=sync, 'c'=scalar, 'g'=gpsimd."""
    nc = bass.Bass()

    def _eng(ch):
        return {"s": nc.sync, "c": nc.scalar, "g": nc.gpsimd}[ch]
    # imgpad planes (bf16, plane-contiguous):
    #   plane 0 (L): L[r, x] = P[r-1, x-1]   (img at [2:130, 2:130])
    #   plane 1 (U): U[r, x] = P[r-1, x]     (img at [2:130, 1:129])
    #   plane 2 (D): D[r, x] = P[r,   x]     (img at [1:129, 1:129])
    img_d = nc.dram_tensor("imgpad", [3, 64, 133, 132], BF16, kind="ExternalInput")
    # dpad3[y, i, x] = Pd[y+i, x]  (padded depth rows, f32)
    dep_d = nc.dram_tensor("dpad3", [128, 3, 130], F32, kind="ExternalInput")
    wp_d = nc.dram_tensor("wp", [128, 4, 64], BF16, kind="ExternalInput")
    w0_d = nc.dram_tensor("w0", [64, 64], BF16, kind="ExternalInput")
    sel_d = nc.dram_tensor("sel", [128, 128], F32, kind="ExternalInput")
    bias_d = nc.dram_tensor("bias", [128, 1], F32, kind="ExternalInput")
    out_d = nc.dram_tensor("out", [128, 2, 4096], BF16, kind="ExternalOutput")

    with tile.TileContext(nc) as tc:
        with (
            tc.tile_pool(name="singles", bufs=1) as singles,
            tc.tile_pool(name="dram", bufs=1, space="DRAM") as drampool,
            tc.tile_pool(name="repp", bufs=1, space="PSUM") as repp,
            tc.tile_pool(name="outps", bufs=3, space="PSUM") as outps,
            tc.tile_pool(name="mods", bufs=4) as mods,
            tc.tile_pool(name="outsb", bufs=2) as outsb,
        ):
            e1, e2, e3 = _eng(rings[0]), _eng(rings[1]), _eng(rings[2])
            # ---- depth + small operands on ring e1 first
            Dt = singles.tile([128, 3, 130], F32)
            e1.dma_start(out=Dt[:], in_=dep_d[:])
            wp_sb = singles.tile([128, 4, 64], BF16)
            w0_sb = singles.tile([64, 64], BF16)
            sel_sb = singles.tile([128, 128], F32)
            bias_sb = singles.tile([128, 1], F32)
            e1.dma_start(out=wp_sb[:], in_=wp_d[:])
            e1.dma_start(out=w0_sb[:], in_=w0_d[:])
            e1.dma_start(out=sel_sb[:], in_=sel_d[:])
            e1.dma_start(out=bias_sb[:], in_=bias_d[:])

            # ---- image stacks.
            # Ta[p<64]  = L rows 32b+1.. : P[32b+r, x-1] (j=1 view at x0=2)
            # Ta[p>=64] = U rows 32b+1.. : P[32b+r, x]   (j=2 view at x0=2)
            # Tb[p<64]  = U rows 32b+1.. : P[32b+r, x]   (k3/k0 at x0=0)
            # Tb[p>=64] = D rows 32b+1.. : P[32b+r+1, x] (k6 at x0=0)
            Ta = singles.tile([128, 4, 34, 132], BF16, name="Ta")
            Tb = singles.tile([128, 4, 34, 132], BF16, name="Tb")
            for b in range(4):
                nc.sync.dma_start(
                    out=Ta[:, b],
                    in_=img_d[0:2, :, 32 * b + 1 : 32 * b + 35, :].rearrange(
                        "h c r x -> (h c) r x"
                    ),
                )
                eng = nc.sync if b % 2 else e2
                eng.dma_start(
                    out=Tb[:, b],
                    in_=img_d[1:3, :, 32 * b + 1 : 32 * b + 35, :].rearrange(
                        "h c r x -> (h c) r x"
                    ),
                )

            # ---- sim (k=1..8) in [y, k, x]: exp(-a*|Pd[y+i,x+j] - Pd[y,x]|)
            simf = singles.tile([128, 8, 128], F32)
            for k in range(1, 9):
                i, j = k // 3, k % 3
                nc.vector.tensor_tensor(
                    out=simf[:, k - 1, :],
                    in0=Dt[:, i, j : j + 128],
                    in1=Dt[:, 0, 0:128],
                    op=mybir.AluOpType.subtract,
                )
            nc.scalar.activation(
                out=simf[:, :, :],
                in_=simf[:, :, :],
                func=mybir.ActivationFunctionType.Abs,
                scale=ALPHA,
            )
            sim_bf = singles.tile([128, 8, 128], BF16)
            nc.scalar.activation(
                out=sim_bf[:, :, :],
                in_=simf[:, :, :],
                func=mybir.ActivationFunctionType.Exp,
                scale=-1.0,
            )
            # spw rows (bf16 pixel pairs packed in fp32 words):
            #   32g+0 = sim for lower half, 32g+1 = upper half
            #   g<3: k = 3g+1+m ; g=3: k = 3, 6
            spw = singles.tile([128, 8192], F32)
            spw_bf = spw[:].bitcast(BF16)  # [128, 16384]
            sim_dram = drampool.tile([8, 16384], BF16)
            e1.dma_start(
                out=sim_dram[:].rearrange("k (y x) -> y k x", y=128),
                in_=sim_bf[:],
            )
            for g in range(4):
                if g < 3:
                    src = sim_dram[3 * g : 3 * g + 2, :]  # k = 3g+1, 3g+2
                else:
                    src = sim_dram[2:6:3, :]  # k = 3, 6
                eng = [e1, e3, e1, e3][g]
                eng.dma_start(out=spw_bf[32 * g : 32 * g + 2, :], in_=src)

            # ---- main loop: 32 tiles of 4 image rows (512 px each) ----
            for t in range(ntiles):
                b = t // 8
                yl = 4 * t - 32 * b
                even = t % 2 == 0
                cpos = 0 if even else 64

                # one full PSUM bank per rep matmul (concurrent row-tiled
                # matmuls must not share a bank); each rep covers 2 tiles
                if even:
                    rep = repp.tile([128, 4, 512], F32, tag="rep", name=f"rep{t}")
                    for g in range(4):
                        if "rep" in skip:
                            continue
                        nc.tensor.matmul(
                            out=rep[:, g, :],
                            lhsT=sel_sb[32 * g : 32 * g + 2, :],
                            rhs=spw[32 * g : 32 * g + 2, 256 * t : 256 * t + 512],
                            start=True,
                            stop=True,
                            tile_position=(32 * g, 0),
                        )
                rep_bf = rep[:].bitcast(BF16)  # [128, 4, 1024]
                ro = 512 * (t % 2)

                m = mods.tile([128, 4, 4, 128], BF16, tag="mod", name=f"m{t}")
                # fused modulate for groups 0..2: in0 rows yl+g+rho, x0=2
                a0 = Ta[:, b, yl : yl + 4, 2:130]
                es = 4 * 34 * 132  # elements per partition in Ta
                base_pat = [
                    [es, 128],
                    [132, 3],
                    [132, 4],
                    [1, 128],
                ]
                if "mod" not in skip:
                    nc.vector.tensor_tensor(
                        out=m[:, 0:3],
                        in0=_ov(a0, base_pat),
                        in1=rep_bf[:, 0:3, ro : ro + 512].rearrange(
                            "p g (r x) -> p g r x", r=4
                        ),
                        op=mybir.AluOpType.mult,
                    )
                    nc.vector.tensor_tensor(
                        out=m[:, 3],
                        in0=Tb[:, b, yl + 1 : yl + 5, 0:128],
                        in1=rep_bf[:, 3, ro : ro + 512].rearrange(
                            "p (r x) -> p r x", r=4
                        ),
                        op=mybir.AluOpType.mult,
                    )
                else:
                    nc.vector.memset(m[:], 0.25)

                if even:
                    out_ps = outps.tile([128, 4, 128], F32, tag="outps", name=f"o{t}")
                out_lo, out_hi = (0, 64) if even else (64, 128)
                for g in range(4):
                    if "main" in skip:
                        continue
                    nc.tensor.matmul(
                        out=out_ps[out_lo:out_hi],
                        lhsT=wp_sb[:, g, :],
                        rhs=m[:, g],
                        start=(g == 0),
                        stop=False,
                        tile_position=(0, cpos),
                    )
                # unmodulated k=0 term: P rows yl.. on Tb lower half
                nc.tensor.matmul(
                    out=out_ps[out_lo:out_hi],
                    lhsT=w0_sb[:, :],
                    rhs=Tb[0:64, b, yl : yl + 4, 0:128],
                    start=("main" in skip),
                    stop=True,
                    tile_position=(0, cpos),
                )

                if not even:
                    if t % 16 == 1:
                        out_sb = outsb.tile(
                            [128, 8, 4, 128], BF16, tag="outsb", name=f"os{t}"
                        )
                    q = (t // 2) % 8
                    nc.scalar.activation(
                        out=out_sb[:, q, :, :],
                        in_=out_ps[:],
                        func=mybir.ActivationFunctionType.Identity,
                        bias=bias_sb[:, 0:1],
                        scale=1.0,
                    )
                    if t % 16 == 15:
                        nc.sync.dma_start(
                            out=out_d[:, t // 16, :],
                            in_=out_sb[:].rearrange("p q r x -> p (q r x)"),
                        )

    if split_waits:
        _split_excess_waits(nc)
    return nc


_NC_CACHE = None


def _get_nc():
    global _NC_CACHE
    if _NC_CACHE is None:
        _NC_CACHE = build_bass()
    return _NC_CACHE


def _prep_operands(weight, bias):
    wtk = weight.reshape(64, 64, 9)  # w[o, c, k]
    wp = np.zeros((128, 4, 64), BF)
    for g in range(3):
        wp[0:64, g, :] = wtk[:, :, 3 * g + 1].T.astype(BF)  # lower = j=1
        wp[64:128, g, :] = wtk[:, :, 3 * g + 2].T.astype(BF)  # upper = j=2
    wp[0:64, 3, :] = wtk[:, :, 3].T.astype(BF)  # lower = k3
    wp[64:128, 3, :] = wtk[:, :, 6].T.astype(BF)  # upper = k6
    w0 = np.ascontiguousarray(wtk[:, :, 0].T).astype(BF)  # [c, o]
    sel = np.zeros((128, 128), np.float32)
    for g in range(4):
        sel[32 * g, 0:64] = 1.0
        sel[32 * g + 1, 64:128] = 1.0
    bias2 = np.concatenate([bias, bias]).reshape(128, 1).astype(np.float32)
    return wp, w0, sel, bias2


def _prep_image(img_bf):
    """img_bf: [64, 128, 128] bf16 -> padded shifted planes [3, 64, 133, 132]."""
    Z = np.zeros((3, 64, 133, 132), BF)
    Z[0, :, 2:130, 2:130] = img_bf  # L: P[r-1, x-1]
    Z[1, :, 2:130, 1:129] = img_bf  # U: P[r-1, x]
    Z[2, :, 1:129, 1:129] = img_bf  # D: P[r, x]
    return Z


def _prep_depth(dep):
    """dep: [128, 128] f32 -> dpad3[y, i, x] = Pd[y+i, x], [128, 3, 130]."""
    Pd = np.zeros((131, 130), np.float32)
    Pd[1:129, 1:129] = dep
    out = np.empty((128, 3, 130), np.float32)
    for i in range(3):
        out[:, i, :] = Pd[i : i + 128, :]
    return out


def kernel(image, depth, weight, bias, **kwargs):
    image = np.asarray(image, dtype=np.float32)
    depth = np.ascontiguousarray(np.asarray(depth, dtype=np.float32))
    weight = np.ascontiguousarray(np.asarray(weight, dtype=np.float32))
    bias = np.ascontiguousarray(np.asarray(bias, dtype=np.float32))

    B = image.shape[0]
    assert B == N_CORES, f"expected batch {N_CORES}, got {B}"

    wp, w0, sel, bias2 = _prep_operands(weight, bias)
    image_bf = image.astype(BF)

    global _last_in_maps
    nc = _get_nc()
    in_maps = [
        {
            "imgpad": _prep_image(image_bf[b]),
            "dpad3": _prep_depth(depth[b, 0]),
            "wp": wp,
            "w0": w0,
            "sel": sel,
            "bias": bias2,
        }
        for b in range(B)
    ]
    _last_in_maps = in_maps
    res = run_bass_kernel_spmd(nc, in_maps, core_ids=list(range(N_CORES)))
    # out[p, B2, q, r, x]: p = h*64 + c (h=0 even tile, h=1 odd);
    # image row = 64*B2 + 8*q + 4*h + r
    outs = []
    for r in res.results:
        v = r["out"].reshape(2, 64, 2, 8, 4, 128)  # h c B2 q r x
        full = np.transpose(v, (1, 2, 3, 0, 4, 5)).reshape(64, 128, 128)
        outs.append(full)
    return np.stack(outs, axis=0).astype(np.float32)
